# revision 4
# baseline (speedup 1.0000x reference)
"""Distributed windowed-attention kernel on 8 Trainium2 NeuronCores.

Data-parallel sharding over (batch, depth-half): each core processes a
(64, 32, 64, 64) sub-volume of whole 4x4x4 windows independently; small
params are replicated. Host-side weight folding removes device passes:
the LN affine (g, b) and the attention scale dh^-0.5 are folded into the
QKV weights, and bias adds are skipped when the folded biases are exactly
zero. Window dims stay implicit in einsums so XLA fuses the window
permutation into matmul operand layouts instead of materializing permuted
copies. Softmax uses unscaled exp/sum (sim is bounded ~|3|, so the
max-subtraction passes are unnecessary). Attention matmuls run in bf16
with f32 accumulation (rel err ~5e-3, well inside the 2e-2 gate)."""
import numpy as np

DIM = 64
HEADS = 2
DH = 32
WSZ = 4
EPS = 1e-5
B0, D, H, W = 4, 64, 64, 64
NCORES = 8

_cache = {}


def _build(devices, has_qb, has_ob):
    import jax
    import jax.numpy as jnp
    from jax.sharding import Mesh, PartitionSpec as P
    from jax.experimental.shard_map import shard_map

    mesh = Mesh(np.array(devices), ("i",))
    w = WSZ

    def per_shard(xsh, qw, qb, ow, ob, ps):
        x = xsh[0]
        C, Ds, Hh, Ww = x.shape
        nd, nh, nw = Ds // w, Hh // w, Ww // w
        cdt = jnp.bfloat16
        f32 = jnp.float32
        mean = jnp.mean(x, axis=0, keepdims=True)
        var = jnp.mean(jnp.square(x - mean), axis=0, keepdims=True)
        xn = (x - mean) * jax.lax.rsqrt(var + EPS)
        xw = xn.reshape(C, nd, w, nh, w, nw, w)
        xw = xw + ps[None, None, :, None, :, None, :]
        xw = xw.astype(cdt)
        # c: channel, d/h/w: window-block indices, i/j/k: in-window offsets
        qkv = jnp.einsum('oc,cdihjwk->odihjwk', qw.astype(cdt), xw,
                         preferred_element_type=f32)
        if has_qb:
            qkv = qkv + qb[:, None, None, None, None, None, None]
        qkv = qkv.reshape(3, HEADS, DH, nd, w, nh, w, nw, w).astype(cdt)
        q, k, v = qkv[0], qkv[1], qkv[2]  # scale pre-folded into q weights
        sim = jnp.einsum('aedihjwk,aedxhywz->adhwijkxyz', q, k,
                         preferred_element_type=f32)
        # unsafe softmax: sim is bounded (|sim| < ~6), skip max subtraction
        e = jnp.exp(sim.reshape(HEADS, nd, nh, nw, w, w, w, w * w * w))
        attn = e * jax.lax.reciprocal(jnp.sum(e, axis=-1, keepdims=True))
        attn = attn.astype(cdt).reshape(HEADS, nd, nh, nw, w, w, w, w, w, w)
        out = jnp.einsum('adhwijkxyz,aedxhywz->aedihjwk', attn, v,
                         preferred_element_type=f32)
        out = out.reshape(HEADS * DH, nd, w, nh, w, nw, w)
        out = jnp.einsum('oc,cdihjwk->odihjwk', ow.astype(cdt), out.astype(cdt),
                         preferred_element_type=f32)
        if has_ob:
            out = out + ob[:, None, None, None, None, None, None]
        out = out.reshape(C, Ds, Hh, Ww).astype(f32)
        return out[None]

    f = shard_map(
        per_shard,
        mesh=mesh,
        in_specs=(P("i"), P(), P(), P(), P(), P()),
        out_specs=P("i"),
    )
    return jax.jit(f)


def kernel(x, g, b, qkv_w, out_w, out_b, pse):
    import jax
    import jax.numpy as jnp

    x = np.asarray(x, dtype=np.float32)
    g = np.asarray(g, dtype=np.float32).reshape(-1)
    b = np.asarray(b, dtype=np.float32).reshape(-1)
    qkv_w = np.asarray(qkv_w, dtype=np.float32)
    out_w = np.asarray(out_w, dtype=np.float32)
    out_b = np.asarray(out_b, dtype=np.float32)
    pse = np.asarray(pse, dtype=np.float32)

    # Fold LN affine into QKV weights, and the attention scale into the
    # q-slice of those weights (q rows are 0:HEADS*DH).
    scale = np.float32(DH ** -0.5)
    qkv_w_eff = qkv_w * g[None, :]
    qkv_b_eff = qkv_w @ b
    qkv_w_eff[: HEADS * DH] *= scale
    qkv_b_eff[: HEADS * DH] *= scale

    has_qb = bool(np.any(qkv_b_eff != 0))
    has_ob = bool(np.any(out_b != 0))

    xs = x.reshape(B0, DIM, 2, D // 2, H, W).transpose(0, 2, 1, 3, 4, 5)
    xs = np.ascontiguousarray(xs.reshape(NCORES, DIM, D // 2, H, W))

    devices = tuple(jax.devices()[:NCORES])
    key = (devices, has_qb, has_ob)
    fn = _cache.get(key)
    if fn is None:
        fn = _build(devices, has_qb, has_ob)
        _cache[key] = fn

    out = fn(
        jnp.asarray(xs),
        jnp.asarray(qkv_w_eff),
        jnp.asarray(qkv_b_eff),
        jnp.asarray(out_w),
        jnp.asarray(out_b),
        jnp.asarray(pse),
    )
    out = np.asarray(out)
    out = out.reshape(B0, 2, DIM, D // 2, H, W).transpose(0, 2, 1, 3, 4, 5)
    return np.ascontiguousarray(out.reshape(B0, DIM, D, H, W), dtype=np.float32)

